# revision 1
# baseline (speedup 1.0000x reference)
"""Trainium2 Bass kernel: fused residual-add + RMSNorm + local (sliding-window)
attention + output projection, sharded over 8 NeuronCores.

Sharding: 8 cores = (batch 4) x (sequence halves 2). Each core owns 2048
tokens of one batch row plus a 64-token halo of keys/values from the
preceding tokens (zeros at sequence start).

All matmuls use K=128 contraction (sequences of K<128 matmuls misbehave on
this toolchain): per-head scores contract over a full 128-feature tile (two
heads) with the other head's query features zeroed; PV contracts over a full
128-key window using phase-shifted copies of v (built with SBUF->SBUF DMA).
"""

import sys

for _p in ("/opt/trn_rl_repo", "/opt/pypackages"):
    if _p not in sys.path:
        sys.path.insert(0, _p)

import numpy as np

import concourse.bacc as bacc
import concourse.bass as bass
import concourse.mybir as mybir
import concourse.tile as tile
from concourse.bass_utils import run_bass_kernel_spmd
from concourse.masks import make_identity

F32 = mybir.dt.float32
F16 = mybir.dt.float16

B, S, D = 4, 4096, 1024
H, DH, C = 16, 64, 64
TOK = 2048          # owned tokens per core
TH = 2176           # 64 zero-pad + 64 halo + 2048 owned
NT = TH // 128      # 17 token tiles
NEG = -30000.0
LN64 = float(np.log(64.0))
EPS = 1e-5

BLOCKS = [(0, 512), (512, 512), (1024, 512), (1536, 512), (2048, 128)]


def _chunks_of_block(b):
    t0, nb = BLOCKS[b]
    return [c for c in range(32) if t0 <= 128 + 64 * c < t0 + nb]


def _out_tiles_of_block(b):
    return sorted({(c + 2) // 2 for c in _chunks_of_block(b)})


def build_nc(stage=3, nblocks=5):
    nc = bacc.Bacc("TRN2", target_bir_lowering=False, debug=False)

    hid_d = nc.dram_tensor("hid", [TH, D], F32, kind="ExternalInput").ap()
    rin_d = nc.dram_tensor("rin", [TH, D], F32, kind="ExternalInput").ap()
    wqk_d = nc.dram_tensor("wqk", [D, 2 * D], F16, kind="ExternalInput").ap()
    wv_d = nc.dram_tensor("wv", [D, D], F16, kind="ExternalInput").ap()
    wo_d = nc.dram_tensor("wo", [D, D], F16, kind="ExternalInput").ap()
    # masks[p, m, 512]: m=0: chunk-0 mask, m=1: band mask (8 heads x 64 q)
    msk_d = nc.dram_tensor("masks", [128, 2, 512], F32, kind="ExternalInput").ap()

    out_d = nc.dram_tensor("out", [TOK, D], F32, kind="ExternalOutput").ap()
    res_d = nc.dram_tensor("res", [TOK, D], F32, kind="ExternalOutput").ap()

    hid_t = hid_d.rearrange("(t p) d -> t p d", p=128)
    rin_t = rin_d.rearrange("(t p) d -> t p d", p=128)
    out_t = out_d.rearrange("(t p) d -> t p d", p=128)
    res_t = res_d.rearrange("(t p) d -> t p d", p=128)

    from contextlib import ExitStack
    with tile.TileContext(nc) as tc, ExitStack() as ctx:
        singles = ctx.enter_context(tc.tile_pool(name="singles", bufs=1))
        io = ctx.enter_context(tc.tile_pool(name="io", bufs=2))
        nrm = ctx.enter_context(tc.tile_pool(name="nrm", bufs=3))
        xp = ctx.enter_context(tc.tile_pool(name="xp", bufs=2))
        xtp = ctx.enter_context(tc.tile_pool(name="xtp", bufs=2))
        qtp = ctx.enter_context(tc.tile_pool(name="qtp", bufs=2))
        ktp = ctx.enter_context(tc.tile_pool(name="ktp", bufs=2))
        vp = ctx.enter_context(tc.tile_pool(name="vp", bufs=2))
        vp1 = ctx.enter_context(tc.tile_pool(name="vp1", bufs=1))
        att = ctx.enter_context(tc.tile_pool(name="att", bufs=3))
        rcp = ctx.enter_context(tc.tile_pool(name="rcp", bufs=3))
        ybp = ctx.enter_context(tc.tile_pool(name="ybp", bufs=2))
        ytp = ctx.enter_context(tc.tile_pool(name="ytp", bufs=2))
        obp = ctx.enter_context(tc.tile_pool(name="obp", bufs=2))
        pp = ctx.enter_context(tc.tile_pool(name="pp", bufs=2, space="PSUM"))
        scp = ctx.enter_context(tc.tile_pool(name="scp", bufs=2, space="PSUM"))
        ypp = ctx.enter_context(tc.tile_pool(name="ypp", bufs=1, space="PSUM"))

        # ---- constants / weights ----
        wqk_sb = singles.tile([128, 8, 2 * D], F16)
        nc.sync.dma_start(wqk_sb[:], wqk_d.rearrange("(ko ki) m -> ki ko m", ki=128))
        wv_sb = singles.tile([128, 8, D], F16)
        nc.sync.dma_start(wv_sb[:], wv_d.rearrange("(ko ki) m -> ki ko m", ki=128))
        wo_sb = singles.tile([128, 8, D], F16)
        nc.sync.dma_start(wo_sb[:], wo_d.rearrange("(ko ki) m -> ki ko m", ki=128))
        msk_sb = singles.tile([128, 2, 512], F32)
        nc.sync.dma_start(msk_sb[:], msk_d)
        ident = singles.tile([128, 128], F16)
        make_identity(nc, ident[:])
        eps_sb = singles.tile([128, 1], F32)
        nc.vector.memset(eps_sb[:], EPS)
        inv_all = singles.tile([128, NT], F32)

        kT_prev = None
        v_prev = None

        for b, (t0, nb) in enumerate(BLOCKS[:nblocks]):
            ntile = nb // 128
            xT_b = xtp.tile([128, 8, 512], F16, tag="xT")

            # ---- norm + transpose for this block's token tiles ----
            for i in range(ntile):
                t = t0 // 128 + i
                ht = io.tile([128, D], F32, tag="hid")
                nc.sync.dma_start(ht[:], hid_t[t])
                rt = io.tile([128, D], F32, tag="rin")
                nc.sync.dma_start(rt[:], rin_t[t])
                nc.vector.tensor_add(ht[:], ht[:], rt[:])  # res
                if t >= 1:
                    nc.sync.dma_start(res_t[t - 1], ht[:])
                # RMS stats: mean(res^2) = var + mean^2
                stats = nrm.tile([128, 2, 6], F32, tag="stats")
                for g in range(2):
                    nc.vector.bn_stats(stats[:, g, :], ht[:, g * 512:(g + 1) * 512])
                mv = nrm.tile([128, 2], F32, tag="mv")
                nc.vector.bn_aggr(mv[:], stats[:])
                ms = nrm.tile([128, 1], F32, tag="ms")
                nc.vector.tensor_mul(ms[:], mv[:, 0:1], mv[:, 0:1])
                nc.vector.tensor_add(ms[:], ms[:], mv[:, 1:2])
                lnm = nrm.tile([128, 1], F32, tag="lnm")
                nc.scalar.activation(lnm[:], ms[:], mybir.ActivationFunctionType.Ln,
                                     bias=eps_sb[:])
                nc.scalar.activation(inv_all[:, t:t + 1], lnm[:],
                                     mybir.ActivationFunctionType.Exp, scale=-0.5)
                x16 = xp.tile([128, D], F16, tag="x16")
                nc.vector.tensor_scalar_mul(x16[:], ht[:], inv_all[:, t:t + 1])
                for kt in range(8):
                    ps = pp.tile([128, 128], F16, tag="tr")
                    nc.tensor.transpose(ps[:], x16[:, kt * 128:(kt + 1) * 128], ident[:])
                    nc.vector.tensor_copy(xT_b[:, kt, i * 128:(i + 1) * 128], ps[:])

            # ---- q/k projection (feature-major out) ----
            # qTz_e: even heads' features at partitions 0:64, zeros at 64:128
            # qTz_o: odd heads' features at partitions 64:128, zeros at 0:64
            qTz_e = qtp.tile([128, 8, 512], F16, tag="qTe")
            qTz_o = qtp.tile([128, 8, 512], F16, tag="qTo")
            nc.vector.memset(qTz_e[64:128, :, :], 0.0)
            nc.vector.memset(qTz_o[0:64, :, :], 0.0)
            kT_b = ktp.tile([128, 8, 576], F16, tag="kT")
            if b > 0:
                nc.vector.tensor_copy(kT_b[:, :, 0:64], kT_prev[:, :, 512:576])
            for mt in range(16):
                ps = pp.tile([128, 512], F32, tag="mm")
                for kt in range(8):
                    nc.tensor.matmul(ps[:, :nb],
                                     wqk_sb[:, kt, mt * 128:(mt + 1) * 128],
                                     xT_b[:, kt, :nb],
                                     start=(kt == 0), stop=(kt == 7))
                if mt < 8:
                    nc.scalar.copy(qTz_e[0:64, mt, :nb], ps[0:64, :nb])
                    nc.scalar.copy(qTz_o[64:128, mt, :nb], ps[64:128, :nb])
                else:
                    nc.scalar.copy(kT_b[:, mt - 8, 64:64 + nb], ps[:, :nb])

            # ---- v projection (token-major, head-interleaved with ones col) ----
            v_b = vp.tile([128, 5, 16 * 65], F16, tag="v")
            if b > 0:
                nc.vector.tensor_copy(v_b[:, 0, :], v_prev[:, 4, :])
            for i in range(ntile):
                vslot = v_b[:, i + 1, :].rearrange("p (h e) -> p h e", e=65)
                for nh in range(2):
                    ps = pp.tile([128, 512], F32, tag="mm")
                    for kt in range(8):
                        nc.tensor.matmul(ps[:],
                                         xT_b[:, kt, i * 128:(i + 1) * 128],
                                         wv_sb[:, kt, nh * 512:(nh + 1) * 512],
                                         start=(kt == 0), stop=(kt == 7))
                    nc.scalar.copy(vslot[:, nh * 8:(nh + 1) * 8, 0:64],
                                   ps[:].rearrange("p (h e) -> p h e", e=64))
                nc.vector.memset(vslot[:, :, 64:65], 1.0)

            # ---- phase-1 v tiles (for even chunks) via SBUF->SBUF DMA ----
            # v1 slot s covers tokens [64+128m, 192+128m), m = 4b-1+s
            cs = _chunks_of_block(b)
            ms_needed = sorted({c // 2 for c in cs if c % 2 == 0})
            v1_b = vp1.tile([128, 4, 16 * 65], F16, tag="v1")
            for m in ms_needed:
                s = m - (4 * b - 1)
                s0 = m - 4 * b + 1      # v_b slot holding global tile m
                nc.sync.dma_start(v1_b[0:64, s, :], v_b[64:128, s0, :])
                nc.sync.dma_start(v1_b[64:128, s, :], v_b[0:64, s0 + 1, :])

            # ---- attention + output projection per 128-token out-tile ----
            for t in _out_tiles_of_block(b):
                yblk = ybp.tile([128, D], F16, tag="yblk")
                if stage == 1:
                    nc.vector.memset(yblk[:], 0.25)
                for hb in range(2 if stage >= 2 else 0):
                    y_ps = ypp.tile([128, 2, 512], F32, tag="y")
                    for which in range(2):
                        c = 2 * t - 2 + which
                        ko = 128 + 64 * c - t0
                        qo = 128 + 64 * c - t0
                        sc_ps = scp.tile([128, 512], F32, tag="sc")
                        for h2 in range(8):
                            h = hb * 8 + h2
                            qTz = qTz_e if h % 2 == 0 else qTz_o
                            nc.tensor.matmul(
                                sc_ps[:, h2 * 64:(h2 + 1) * 64],
                                kT_b[:, h // 2, ko:ko + 128],
                                qTz[:, h // 2, qo:qo + 64],
                                start=True, stop=True)
                        mi = 0 if c == 0 else 1
                        nc.vector.tensor_add(sc_ps[:], sc_ps[:], msk_sb[:, mi, :])
                        expS = att.tile([128, 512], F16, tag="expS")
                        nc.scalar.activation(expS[:], sc_ps[:],
                                             mybir.ActivationFunctionType.Exp)
                        if stage == 2:
                            if which == 0:
                                nc.vector.tensor_copy(
                                    yblk[:, hb * 512:(hb + 1) * 512], expS[:])
                            continue
                        # PV: contract over the 128-key window
                        if c % 2 == 0:
                            vt = v1_b[:, c // 2 - (4 * b - 1), :]
                        else:
                            vt = v_b[:, (c + 1) // 2 - 4 * b + 1, :]
                        vtile = vt.rearrange("p (h e) -> p h e", e=65)
                        for h2 in range(8):
                            h = hb * 8 + h2
                            oap = y_ps[which * 64:(which + 1) * 64, h2 // 4,
                                       (h2 % 4) * 65:(h2 % 4) * 65 + 65]
                            nc.tensor.matmul(
                                oap, expS[:, h2 * 64:(h2 + 1) * 64],
                                vtile[:, h, :],
                                start=True, stop=True,
                                tile_position=(0, which * 64))
                    if stage == 2:
                        continue
                    # normalize: per-query reciprocal of denominator column
                    ybank = y_ps[:, :, 0:260].rearrange("p b (h e) -> p b h e",
                                                        e=65)
                    rc = rcp.tile([128, 2, 4, 1], F32, tag="rc")
                    nc.vector.reciprocal(rc[:], ybank[:, :, :, 64:65])
                    ydst = yblk[:, hb * 512:(hb + 1) * 512].rearrange(
                        "p (b h e) -> p b h e", b=2, h=4)
                    nc.vector.tensor_mul(ydst, ybank[:, :, :, 0:64],
                                         rc[:].to_broadcast([128, 2, 4, 64]))
                # yT + output projection
                yT_t = ytp.tile([128, 8, 128], F16, tag="yT")
                for kt in range(8):
                    ps = pp.tile([128, 128], F16, tag="tr")
                    nc.tensor.transpose(ps[:], yblk[:, kt * 128:(kt + 1) * 128], ident[:])
                    nc.vector.tensor_copy(yT_t[:, kt, :], ps[:])
                for nh in range(2):
                    ps = pp.tile([128, 512], F32, tag="mm")
                    for kt in range(8):
                        nc.tensor.matmul(ps[:], yT_t[:, kt, :],
                                         wo_sb[:, kt, nh * 512:(nh + 1) * 512],
                                         start=(kt == 0), stop=(kt == 7))
                    osb = obp.tile([128, 512], F32, tag="osb")
                    nc.scalar.copy(osb[:], ps[:])
                    nc.sync.dma_start(out_t[t - 1][:, nh * 512:(nh + 1) * 512], osb[:])

            kT_prev = kT_b
            v_prev = v_b

    nc.compile()
    return nc


def _build_masks(seq_start: bool) -> np.ndarray:
    j = np.arange(128)[:, None]   # key pos in window
    i = np.arange(64)[None, :]    # query pos in chunk
    band = (j >= i) & (j <= i + 64)
    m0 = band & (j >= 64)         # chunk 0 at sequence start

    def vals(m):
        return np.where(m, -LN64, NEG).astype(np.float32)

    out = np.empty((128, 2, 512), np.float32)
    out[:, 0, :] = np.tile(vals(m0 if seq_start else band), (1, 8))
    out[:, 1, :] = np.tile(vals(band), (1, 8))
    return out


_NC = None


def kernel(hidden_states, residual, norm_weight, w_qkv, w_out, trace=False):
    global _NC
    if _NC is None:
        _NC = build_nc()
    nc = _NC

    hidden_states = np.asarray(hidden_states, np.float32)
    residual = np.asarray(residual, np.float32)
    norm_weight = np.asarray(norm_weight, np.float32)
    w_qkv = np.asarray(w_qkv, np.float32)
    w_out = np.asarray(w_out, np.float32)

    wqk = (norm_weight[:, None] * w_qkv[:, :2 * D]).copy()
    wqk[:, :D] *= DH ** -0.5
    wqk16 = wqk.astype(np.float16)
    wv16 = (norm_weight[:, None] * w_qkv[:, 2 * D:]).astype(np.float16)
    wo16 = w_out.astype(np.float16)

    in_maps = []
    for core in range(8):
        b, s = core // 2, core % 2
        hid = np.zeros((TH, D), np.float32)
        rin = np.zeros((TH, D), np.float32)
        if s == 1:
            hid[64:128] = hidden_states[b, TOK - 64:TOK]
            rin[64:128] = residual[b, TOK - 64:TOK]
        hid[128:] = hidden_states[b, s * TOK:(s + 1) * TOK]
        rin[128:] = residual[b, s * TOK:(s + 1) * TOK]
        in_maps.append({
            "hid": hid, "rin": rin,
            "wqk": wqk16, "wv": wv16, "wo": wo16,
            "masks": _build_masks(seq_start=(s == 0)),
        })

    r = run_bass_kernel_spmd(nc, in_maps, list(range(8)), trace=trace)
    if trace:
        kernel.last_exec_ns = r.exec_time_ns
        kernel.last_results = r
    kernel.last_in_maps = in_maps

    out = np.empty((B, S, D), np.float32)
    res = np.empty((B, S, D), np.float32)
    for core in range(8):
        b, s = core // 2, core % 2
        out[b, s * TOK:(s + 1) * TOK] = r.results[core]["out"]
        res[b, s * TOK:(s + 1) * TOK] = r.results[core]["res"]
    return out, res


def bench(in_maps, iters=20):
    """Steady-state wall time per execution of the compiled NEFF across the
    8 cores (includes PJRT/axon dispatch overhead; upper bound on HW time)."""
    import time

    import jax
    from jax.experimental.shard_map import shard_map
    from jax.sharding import Mesh, NamedSharding, PartitionSpec

    from concourse import bass2jax, mybir as _mb

    nc = _NC
    bass2jax.install_neuronx_cc_hook()
    partition_name = nc.partition_id_tensor.name if nc.partition_id_tensor else None

    in_names, out_names, out_avals, zero_outs = [], [], [], []
    for alloc in nc.m.functions[0].allocations:
        if not isinstance(alloc, _mb.MemoryLocationSet):
            continue
        name = alloc.memorylocations[0].name
        if alloc.kind == "ExternalInput":
            if name != partition_name:
                in_names.append(name)
        elif alloc.kind == "ExternalOutput":
            shape = tuple(alloc.tensor_shape)
            dtype = _mb.dt.np(alloc.dtype)
            out_names.append(name)
            out_avals.append(jax.core.ShapedArray(shape, dtype))
            zero_outs.append(np.zeros(shape, dtype))
    n_params = len(in_names)
    n_outs = len(out_avals)
    all_in = list(in_names) + list(out_names)
    if partition_name is not None:
        all_in.append(partition_name)
    donate = tuple(range(n_params, n_params + n_outs))

    def _body(*args):
        operands = list(args)
        if partition_name is not None:
            operands.append(bass2jax.partition_id_tensor())
        return tuple(bass2jax._bass_exec_p.bind(
            *operands,
            out_avals=tuple(out_avals),
            in_names=tuple(all_in),
            out_names=tuple(out_names),
            lowering_input_output_aliases=(),
            sim_require_finite=True,
            sim_require_nnan=True,
            nc=nc,
        ))

    devices = jax.devices()[:8]
    mesh = Mesh(np.asarray(devices), ("core",))
    in_specs = (PartitionSpec("core"),) * (n_params + n_outs)
    out_specs = (PartitionSpec("core"),) * n_outs
    sharded = jax.jit(
        shard_map(_body, mesh=mesh, in_specs=in_specs, out_specs=out_specs,
                  check_rep=False),
        donate_argnums=donate, keep_unused=True)

    concat_in = [np.concatenate([np.asarray(in_maps[c][n]) for c in range(8)], axis=0)
                 for n in in_names]
    shd = NamedSharding(mesh, PartitionSpec("core"))
    dev_in = [jax.device_put(a, shd) for a in concat_in]
    zeros_np = [np.zeros((8 * z.shape[0], *z.shape[1:]), z.dtype) for z in zero_outs]

    times = []
    outs = None
    for it in range(iters):
        dz = [jax.device_put(z, shd) for z in zeros_np]
        jax.block_until_ready(dz)
        t0 = time.perf_counter()
        outs = sharded(*dev_in, *dz)
        jax.block_until_ready(outs)
        times.append(time.perf_counter() - t0)
    return times, outs

